# revision 1
# baseline (speedup 1.0000x reference)
"""HBond whole-pose scoring on 8 Trainium2 NeuronCores.

Strategy (per sharding hint): data-parallel over poses — one pose per core.
Host does only metadata-derived prep (index compaction, one-hot encodings,
coefficient table gathers — all O(B*MD) sized); each NeuronCore does the
O(N^2) work: the distance-squared plane as a K=5 matmul, per-pair coefficient
/ bound planes as K=6 / K=166 one-hot matmuls (block-pair validity folded into
the lower-bound plane as a +LARGE offset), degree-10 Horner, range masks, and
the full reduction.
"""
import numpy as np

P, B, T = 8, 160, 32
MD, MA = 8, 8
ND, NA = 6, 6
NBT = 20
K = 11
MIN_SEP = 4
LARGE = np.float32(1.0e6)


def _prep_pose(p, coords, block_type, min_bond_sep, n_donH, donH_inds, donH_type,
               n_acc, acc_inds, acc_type, pair_params, pair_polynomials, gp, Dp, Ap):
    f32 = np.float32
    bt = block_type[p]
    c = coords[p].astype(f32)

    # compact donor / acceptor lists (pure index metadata)
    nd = n_donH[bt]                          # [B]
    d_blk = np.repeat(np.arange(B), nd)
    d_sub = np.concatenate([np.arange(n) for n in nd])
    d_atom = d_blk * T + donH_inds[bt[d_blk], d_sub]
    d_type = donH_type[bt[d_blk], d_sub]
    na = n_acc[bt]
    a_blk = np.repeat(np.arange(B), na)
    a_sub = np.concatenate([np.arange(n) for n in na])
    a_atom = a_blk * T + acc_inds[bt[a_blk], a_sub]
    a_type = acc_type[bt[a_blk], a_sub]
    nD, nA_ = len(d_atom), len(a_atom)

    # padded coordinates; pads replicate atom 0 (kept close; masked via zero tables)
    H = np.zeros((Dp, 3), f32); H[:nD] = c[d_atom]; H[nD:] = c[0]
    A = np.zeros((Ap, 3), f32); A[:nA_] = c[a_atom]; A[nA_:] = c[0]

    # coefficient tables with weight and global scale folded in
    w = pair_params[:, :, 2].astype(f32) * f32(gp)
    coef = pair_polynomials.astype(f32) * w[:, :, None]      # [ND,NA,K]
    G = np.zeros((Dp, NA, K), f32)
    G[:nD] = coef[d_type]                                    # [nD,NA,K]

    dmin2 = (pair_params[:, :, 0].astype(f32)) ** 2
    dmax2 = (pair_params[:, :, 1].astype(f32)) ** 2
    Gdmin = np.zeros((Dp, NA), f32); Gdmin[:nD] = dmin2[d_type]
    Gdmax = np.zeros((Dp, NA), f32); Gdmax[:nD] = dmax2[d_type]

    # acceptor one-hots (pad columns all-zero -> dmax plane 0 -> masked)
    Oa = np.zeros((NA, Ap), f32); Oa[a_type, np.arange(nA_)] = 1.0
    # block-pair validity folded into the dmin plane via +LARGE
    sep_ok = (min_bond_sep[p] >= MIN_SEP) & ~np.eye(B, dtype=bool)
    notall = (~sep_ok).astype(f32) * LARGE                   # [B,B]
    Eh = np.zeros((Dp, B), f32); Eh[np.arange(nD), d_blk] = 1.0
    Ea = np.zeros((B, Ap), f32); Ea[a_blk, np.arange(nA_)] = 1.0
    Vd = Eh @ notall                                         # [Dp,B] (tiny host matmul of 0/LARGE rows)
    return H, A, G, Gdmin, Gdmax, Oa, Vd, Ea


def _pose_fn(jnp):
    def f(H, A, G, Gdmin, Gdmax, Oa, Vd, Ea):
        # distance-squared plane via K=5 matmul
        lhs = jnp.concatenate([-2.0 * H, (H * H).sum(1, keepdims=True),
                               jnp.ones((H.shape[0], 1), jnp.float32)], axis=1)
        rhs = jnp.concatenate([A, jnp.ones((A.shape[0], 1), jnp.float32),
                               (A * A).sum(1, keepdims=True)], axis=1)
        s = lhs @ rhs.T                                      # [Dp,Ap]
        s = jnp.maximum(s, 0.0)
        d = jnp.sqrt(s + 1e-12)

        dmin_pl = Gdmin @ Oa + Vd @ Ea                       # [Dp,Ap]
        dmax_pl = Gdmax @ Oa
        m = (s >= dmin_pl) & (s <= dmax_pl)

        E = G[:, :, 0] @ Oa
        for k in range(1, K):
            E = E * d + G[:, :, k] @ Oa
        E = jnp.where(m, E, 0.0)
        return E.sum()
    return f


def kernel(coords, pair_params, pair_polynomials, global_params,
           block_type, min_bond_sep, n_donH, donH_inds, donH_type,
           n_acc, acc_inds, acc_type):
    import jax
    import jax.numpy as jnp

    coords = np.asarray(coords); block_type = np.asarray(block_type)
    min_bond_sep = np.asarray(min_bond_sep)
    n_donH = np.asarray(n_donH); donH_inds = np.asarray(donH_inds)
    donH_type = np.asarray(donH_type)
    n_acc = np.asarray(n_acc); acc_inds = np.asarray(acc_inds)
    acc_type = np.asarray(acc_type)
    pair_params = np.asarray(pair_params)
    pair_polynomials = np.asarray(pair_polynomials)
    gp = float(np.asarray(global_params)[0, 0])

    # common padded sizes across poses -> one compiled program for all cores
    ndon = n_donH[block_type].sum(axis=1)      # [P]
    nacc = n_acc[block_type].sum(axis=1)
    Dp = int(-(-int(ndon.max()) // 128) * 128)
    Ap = int(-(-int(nacc.max()) // 128) * 128)

    preps = [_prep_pose(p, coords, block_type, min_bond_sep, n_donH, donH_inds,
                        donH_type, n_acc, acc_inds, acc_type, pair_params,
                        pair_polynomials, gp, Dp, Ap) for p in range(P)]

    devs = jax.devices()[:P]
    f = jax.jit(_pose_fn(jnp))
    # dispatch one pose per NeuronCore; async dispatch then gather
    futs = []
    for p in range(P):
        args = [jax.device_put(a, devs[p]) for a in preps[p]]
        futs.append(f(*args))
    out = np.array([np.float32(v) for v in futs], dtype=np.float32)
    return out
